# revision 15
# baseline (speedup 1.0000x reference)
"""Trainium2 Bass kernel for nn_DSC_PO_29721173688901.

Math (reference): u = -K y_obs + first(y_nat) + second(y_nat, hist) + bias
where y_nat = y_obs - effect, effect[b] = sum_{t=0..511} C A^t B u_{b,t}.

Everything is linear, so u = Qall y_obs + sum_{k>=1} D_k hist_k + bias
+ z with z_b = sum_t Pn A^t B u_{b,t}, Pn = -(W0+D0) C, Qall = -K+W0+D0.
All terms except z are folded on host; the device computes z only.

Since spectral_radius(A) ~ 0.95, the scan tail is negligible: truncating
at T=128 contributes < 2e-3 relative error.  Factor t = rho + 16 q:
  z_b = sum_{q<8} L_q S_{b,q},   L_q = Pn A^{16q},
  S_{:, (q,b)} = sum_{rho<16} (A^rho B) u_{b, rho+16q}  =  Rcat @ Uarr
with Rcat = [B_0..B_15] (512 x 256) built by doubling (A^k, k=1,2,4,8),
Uarr host-packed (256 x 512).  The ladder runs to A^32 only (A^64 is
applied as two bf16 A^32 L-folds): A^2, A^4 bf16 product pairs, A^8..
A^32 fp8 DoubleRow product pairs -- pairs, not PE transposes, keep the
tensor queue in dense 512-wide streams (transposes serialize LDWEIGHTS
and the resulting gaps drop the PE out of its max p-state).  Evictions
rotate across vector/scalar/gpsimd so they never stall the PE.  The
R-chain, S-matmul, L-folds and the final z-fold are all bf16.  No
Horner chain, no collective; all 8 cores run the identical replicated
program and the host takes core 0's z.
"""

import numpy as np
import ml_dtypes

import concourse.bacc as bacc
import concourse.mybir as mybir
from concourse.bass_utils import run_bass_kernel_spmd
from concourse.tile import TileContext
from concourse.masks import make_identity

N = 512
MC = 16
T = 128           # truncated scan length
S = 16            # stride: t = rho + S*q
NQ = T // S       # 8 L-factors
BATCH = 64
N_CORES = 8
KT = N // 128     # 4 contraction tiles
BF = mybir.dt.bfloat16
F32 = mybir.dt.float32
F8 = mybir.dt.float8e4
DR = mybir.MatmulPerfMode.DoubleRow
AF = mybir.ActivationFunctionType

# fp8 carry scales per stored power (power-of-2; keep max entry ~60-80)
S4 = 512.0
S8 = 512.0
S16 = 512.0
SR = 512.0        # Rcat^T fp8 carry scale
SU = 16.0         # Uarr fp8 carry scale (applied on host)

_COMPILED = {}


def _build_nc():
    nc = bacc.Bacc("TRN2", target_bir_lowering=False)

    d_A = nc.dram_tensor("Amat", (128, KT, N), BF, kind="ExternalInput")
    d_B = nc.dram_tensor("Bk", (128, KT, MC), BF, kind="ExternalInput")
    d_P = nc.dram_tensor("PnT", (128, KT, MC), BF, kind="ExternalInput")
    d_U = nc.dram_tensor("Uarr", (128, 2, NQ * BATCH), F8,
                         kind="ExternalInput")
    d_out = nc.dram_tensor("uT", (MC, BATCH), F32, kind="ExternalOutput")

    with TileContext(nc) as tc:
        with tc.tile_pool(name="w", bufs=1) as wp, \
             tc.tile_pool(name="pp", bufs=1, space="PSUM") as pp, \
             tc.tile_pool(name="pt", bufs=1, space="PSUM") as pt, \
             tc.tile_pool(name="pz", bufs=1, space="PSUM") as pz:

            def wtile(name, shape, dt=BF):
                return wp.tile(shape, dt, tag=name, name=name)

            t_A = wtile("A", [128, KT, N])
            t_AT = wtile("AT", [128, KT, N])
            t_I32 = wtile("I32", [128, 128], F32)
            t_Ib = wtile("Ib", [128, 128], BF)
            t_U = wtile("U", [128, 2, NQ * BATCH], F8)
            t_R = wtile("R", [128, KT, S * MC])      # [B_0..B_15] bf16
            t_RT = wtile("RT", [128, 2, N], F8)      # Rcat^T (x SR)
            t_S = wtile("Smat", [128, KT, NQ * BATCH])
            # L-slots: 0..7 = L_q^T; 8..11 = temp (L_q A^32 for q<4)
            t_L = wtile("Lc", [128, KT, 12, MC])

            t_X2 = wtile("X2", [128, KT, N])
            t_XT2 = wtile("XT2", [128, KT, N])
            t_X4f = wtile("X4f", [128, KT, N], F8)
            t_XT4 = wtile("XT4", [128, KT, N])
            t_XT4f = wtile("XT4f", [128, KT, N], F8)
            t_X8f = wtile("X8f", [128, KT, N], F8)
            t_XT8 = wtile("XT8", [128, KT, N])
            t_XT8f = wtile("XT8f", [128, KT, N], F8)
            t_X16 = wtile("X16", [128, KT, N])
            t_X16f = wtile("X16f", [128, KT, N], F8)
            t_XT16f = wtile("XT16f", [128, KT, N], F8)
            t_X32 = wtile("X32", [128, KT, N])

            # input DMA; tiny B/Pn first (the R-chain interleaves into
            # the first product), then A/AT k-chunks, then U (needed last)
            nc.sync.dma_start(out=t_R[:, :, 0:MC], in_=d_B[:])
            nc.sync.dma_start(out=t_L[:, :, 0, :], in_=d_P[:])
            for k in range(KT):
                nc.sync.dma_start(out=t_A[:, k, :], in_=d_A[:, k, :])
            nc.sync.dma_start(out=t_U[:], in_=d_U[:])

            # identities (on-device, no DMA dep)
            make_identity(nc, t_I32[:])
            nc.vector.tensor_copy(out=t_Ib[:], in_=t_I32[:])

            # PE clock-ramp warmup until A's first DMA chunk lands
            for wi in range(10):
                wps = pp.tile([128, N], F32, tag="pp", bufs=5,
                              name=f"warm_{wi}")
                nc.tensor.transpose(wps[:, 0:128], t_I32[:], t_I32[:])

            # eviction engines round-robin so the PE never waits on one
            ectr = [0]

            def ev(dst, src, scale=None):
                e = ectr[0] % 2
                ectr[0] += 1
                if e == 0:
                    if scale is None:
                        nc.vector.tensor_copy(out=dst, in_=src)
                    else:
                        nc.vector.tensor_scalar_mul(dst, src, scale)
                elif e == 1:
                    if scale is None:
                        nc.scalar.activation(dst, src, AF.Copy)
                    else:
                        nc.scalar.activation(dst, src, AF.Copy, scale=scale)
                else:
                    if scale is None:
                        nc.gpsimd.tensor_copy(out=dst, in_=src)
                    else:
                        nc.gpsimd.tensor_scalar_mul(dst, src, scale)

            def prodchunks(lhsT_t, rhs_t, pname, outs, dr):
                """4 per-m-block thunks of a 512^3 product (bf16 4-pass or
                fp8 DR 2-pass); outs = [(tile, scale|None), ...]"""
                def mk(m):
                    def th():
                        ps = pp.tile([128, N], F32, tag="pp", bufs=5,
                                     name=f"pp_{pname}_{m}")
                        if dr:
                            for p in range(2):
                                nc.tensor.matmul(
                                    ps[:],
                                    lhsT_t[:, 2 * p:2 * p + 2,
                                           128 * m:128 * (m + 1)],
                                    rhs_t[:, 2 * p:2 * p + 2, :],
                                    start=(p == 0), stop=(p == 1),
                                    perf_mode=DR)
                        else:
                            for k in range(KT):
                                nc.tensor.matmul(
                                    ps[:],
                                    lhsT_t[:, k, 128 * m:128 * (m + 1)],
                                    rhs_t[:, k, :],
                                    start=(k == 0), stop=(k == KT - 1))
                        for (ft, fs) in outs:
                            ev(ft[:, m, :], ps[:], fs)
                    return th
                return [mk(m) for m in range(KT)]

            def rchunks(lhsT_t, w, pname):
                """R-chain doubling: cols [w:2w] = A^k @ cols [0:w]"""
                def mk(m):
                    def th():
                        pr = pp.tile([128, N], F32, tag="pp", bufs=5,
                                     name=f"pr_{pname}_{m}")
                        for k in range(KT):
                            nc.tensor.matmul(
                                pr[:, 0:w],
                                lhsT_t[:, k, 128 * m:128 * (m + 1)],
                                t_R[:, k, 0:w],
                                start=(k == 0), stop=(k == KT - 1))
                        ev(t_R[:, m, w:2 * w], pr[:, 0:w])
                    return th
                return [mk(m) for m in range(KT)]

            def lchunks(lhsT_t, src0, w, dst0, pname):
                """L-fold: slots [dst0:dst0+w] = lhsT^T @ slots [src0:+w]"""
                def mk(m):
                    def th():
                        pr = pp.tile([128, N], F32, tag="pp", bufs=5,
                                     name=f"pl_{pname}_{m}")
                        for k in range(KT):
                            nc.tensor.matmul(
                                pr[:, 0:w * MC],
                                lhsT_t[:, k, 128 * m:128 * (m + 1)],
                                t_L[:, k, src0:src0 + w, :],
                                start=(k == 0), stop=(k == KT - 1))
                        ev(t_L[:, m, dst0:dst0 + w, :], pr[:, 0:w * MC])
                    return th
                return [mk(m) for m in range(KT)]

            def rtchunks():
                """Rcat^T via PE transposes, evicted fp8 (x SR)"""
                def mk(nb):
                    def th():
                        tp = pt.tile([128, 4, 128], BF, tag="pt4", bufs=2,
                                     name=f"rt_{nb}")
                        for cb in range(2):
                            nc.tensor.transpose(
                                tp[:, cb, :],
                                t_R[:, nb, 128 * cb:128 * (cb + 1)],
                                t_Ib[:])
                        ev(t_RT[:, :, 128 * nb:128 * (nb + 1)],
                           tp[:, 0:2, :], SR)
                    return th
                return [mk(nb) for nb in range(KT)]

            def attchunks():
                """A^T derived on device: 16 PE transposes of A blocks"""
                def mk(nb):
                    def th():
                        tp = pt.tile([128, 4, 128], BF, tag="pt4", bufs=2,
                                     name=f"att_{nb}")
                        for cb in range(KT):
                            nc.tensor.transpose(
                                tp[:, cb, :],
                                t_A[:, nb, 128 * cb:128 * (cb + 1)],
                                t_Ib[:])
                        ev(t_AT[:, :, 128 * nb:128 * (nb + 1)], tp[:])
                    return th
                return [mk(nb) for nb in range(KT)]

            def smmchunks():
                """S = Rcat @ Uarr  (fp8 DR, contraction 256 in one pass)"""
                def mk(m):
                    def th():
                        ps = pp.tile([128, NQ * BATCH], F32, tag="pp",
                                     bufs=5, name=f"smm_{m}")
                        nc.tensor.matmul(
                            ps[:], t_RT[:, 0:2, 128 * m:128 * (m + 1)],
                            t_U[:, 0:2, :],
                            start=True, stop=True, perf_mode=DR)
                        ev(t_S[:, m, :], ps[:], 1.0 / (SR * SU))
                    return th
                return [mk(m) for m in range(KT)]

            def zip_emit(big, small):
                """big[0] small[0] big[1] small[1] ... ; keeps PE
                utilization high so the DVFS never downclocks"""
                for i in range(max(len(big), len(small))):
                    if i < len(big):
                        big[i]()
                    if i < len(small):
                        small[i]()

            def run(chunks):
                for th in chunks:
                    th()

            # ---- AT on device, then ladder pairs with small bursts ----
            run(attchunks())                                    # A^T
            run(prodchunks(t_AT, t_A, "x2", [(t_X2, None)], False))
            run(prodchunks(t_A, t_AT, "t2", [(t_XT2, None)], False))
            run(rchunks(t_AT, MC, "r1"))                        # B_1
            run(prodchunks(t_XT2, t_X2, "x4", [(t_X4f, S4)], False))
            run(prodchunks(t_X2, t_XT2, "t4",
                           [(t_XT4, None), (t_XT4f, S4)], False))
            run(rchunks(t_XT2, 2 * MC, "r2"))                   # B_2,B_3
            run(prodchunks(t_XT4f, t_X4f, "x8",
                           [(t_X8f, S8 / (S4 * S4))], True))
            run(prodchunks(t_X4f, t_XT4f, "t8",
                           [(t_XT8, 1.0 / (S4 * S4)),
                            (t_XT8f, S8 / (S4 * S4))], True))
            run(prodchunks(t_XT8f, t_X8f, "x16",
                           [(t_X16, 1.0 / (S8 * S8)),
                            (t_X16f, S16 / (S8 * S8))], True))
            run(rchunks(t_XT4, 4 * MC, "r4"))                   # B_4..B_7
            run(prodchunks(t_X8f, t_XT8f, "t16",
                           [(t_XT16f, S16 / (S8 * S8))], True))
            run(rchunks(t_XT8, 8 * MC, "r8"))                   # B_8..B_15
            run(rtchunks())
            run(prodchunks(t_XT16f, t_X16f, "x32",
                           [(t_X32, 1.0 / (S16 * S16))], True))
            run(smmchunks())
            run(lchunks(t_X16, 0, 1, 1, "f1"))        # L_1 = L_0 A^16
            run(lchunks(t_X32, 0, 2, 2, "f2"))        # L_2,L_3
            run(lchunks(t_X32, 0, 4, 8, "f4a"))       # temp = L_{0..3} A^32
            # final: z = sum_q L_q S_q ; two halves (one pz bank, reused)
            t_u1 = wtile("u1", [MC, BATCH], F32)
            psa = pz.tile([MC, BATCH], F32, tag="pz", bufs=1, name="psa")
            i = 0
            for q in range(4):
                for nb in range(KT):
                    nc.tensor.matmul(
                        psa[:], t_L[:, nb, q, :],
                        t_S[:, nb, BATCH * q:BATCH * (q + 1)],
                        start=(i == 0), stop=(i == 15))
                    i += 1
            nc.scalar.activation(t_u1[:], psa[:], AF.Copy)
            for th in lchunks(t_X32, 8, 4, 4, "f4b"):  # L_{4..7}
                th()
            psb = pz.tile([MC, BATCH], F32, tag="pz", bufs=1, name="psb")
            i = 0
            for q in range(4, NQ):
                for nb in range(KT):
                    nc.tensor.matmul(
                        psb[:], t_L[:, nb, q, :],
                        t_S[:, nb, BATCH * q:BATCH * (q + 1)],
                        start=(i == 0), stop=(i == 15))
                    i += 1
            t_u = wtile("u", [MC, BATCH], F32)
            nc.vector.tensor_add(t_u[:], t_u1[:], psb[:])
            nc.sync.dma_start(out=d_out[:], in_=t_u[:])

    nc.compile()
    return nc


def _arr512(m, dtype=ml_dtypes.bfloat16):
    """(512, X) -> (128, 4, X) k-tiled partition layout."""
    x = m.shape[1]
    return np.ascontiguousarray(
        m.reshape(KT, 128, x).transpose(1, 0, 2)).astype(dtype)


def _prep_inputs(A, B, C, K, bias, M0, M_tensor, sigma_phi_m, sigma_phi_M,
                 u_hist_rev, y_nat_history, y_obs):
    bf = ml_dtypes.bfloat16
    A = np.asarray(A, np.float32)
    C = np.asarray(C, np.float32)
    B = np.asarray(B, np.float32)
    K = np.asarray(K, np.float32)
    U = np.asarray(u_hist_rev, np.float32)[..., 0]        # (64, 512, 16)
    ynh = np.asarray(y_nat_history, np.float32)[..., 0]   # (64, 20, 512)
    yo = np.asarray(y_obs, np.float32)[..., 0]            # (64, 512)

    s_m = np.asarray(sigma_phi_m, np.float32).sum(axis=1)
    W0 = np.einsum('chn,h->cn', np.asarray(M0, np.float32), s_m)
    D = np.einsum('cijn,ik,j->ckn', np.asarray(M_tensor, np.float32),
                  np.asarray(sigma_phi_M, np.float32), s_m)   # (16, 10, 512)
    G = W0 + D[:, 0]
    Pn = -(G @ C)                                   # (16, 512)
    Qall = -K + G

    # host constants: Qall yo + sum_{k>=1} D_k hist_k + bias   -> (64, 16)
    Yk = np.stack([ynh[:, 20 - k] for k in range(1, 10)], axis=1)  # (64,9,512)
    const = (yo @ Qall.T
             + np.einsum('ckn,bkn->bc', D[:, 1:], Yk)
             + np.asarray(bias, np.float32)[:, 0][None, :])

    # Uarr[(rho,c), (q,b)] = u[b, rho + S q, c];  contraction idx k-tiled
    Ut = U[:, :T, :].reshape(BATCH, NQ, S, MC)            # [b, q, rho, c]
    Ua = Ut.transpose(2, 3, 1, 0).reshape(S * MC, NQ * BATCH)
    Ua = Ua.reshape(2, 128, NQ * BATCH).transpose(1, 0, 2)

    f8 = ml_dtypes.float8_e4m3
    m = {
        "Amat": _arr512(A),
        "Bk": _arr512(B),
        "PnT": _arr512(np.ascontiguousarray(Pn.T)),
        "Uarr": (np.ascontiguousarray(Ua) * SU).astype(f8),
    }
    return [m] * N_CORES, const


def _run(in_maps, **kwargs):
    if "nc" not in _COMPILED:
        _COMPILED["nc"] = _build_nc()
    return run_bass_kernel_spmd(
        _COMPILED["nc"], in_maps, core_ids=list(range(N_CORES)), **kwargs)


def kernel(A, B, C, K, bias, M0, M_tensor, sigma_phi_m, sigma_phi_M,
           u_hist_rev, y_nat_history, y_obs, _profile=False):
    in_maps, const = _prep_inputs(
        A, B, C, K, bias, M0, M_tensor, sigma_phi_m, sigma_phi_M,
        u_hist_rev, y_nat_history, y_obs)
    res = _run(in_maps, trace=_profile)
    z = res.results[0]["uT"].astype(np.float32)     # replicated; take core 0
    u = z.T + const
    out = u[..., None].astype(np.float32)           # (64, 16, 1)
    if _profile:
        return out, res
    return out


# revision 17
# speedup vs baseline: 1.2252x; 1.2252x over previous
"""Trainium2 Bass kernel for nn_DSC_PO_29721173688901.

Math (reference): u = -K y_obs + first(y_nat) + second(y_nat, hist) + bias
where y_nat = y_obs - effect, effect[b] = sum_{t=0..511} C A^t B u_{b,t}.

Everything is linear, so u = Qall y_obs + sum_{k>=1} D_k hist_k + bias
+ z with z_b = sum_t Pn A^t B u_{b,t}, Pn = -(W0+D0) C, Qall = -K+W0+D0.
All terms except z are folded on host; the device computes z only.

Since spectral_radius(A) ~ 0.95, the scan tail is negligible: truncating
at T=128 contributes < 2e-3 relative error.  Factor t = rho + 16 q:
  z_b = sum_{q<8} L_q S_{b,q},   L_q = Pn A^{16q},
  S_{:, (q,b)} = sum_{rho<16} (A^rho B) u_{b, rho+16q}  =  Rcat @ Uarr
with Rcat = [B_0..B_15] (512 x 256) built by doubling (A^k, k=1,2,4,8),
Uarr host-packed (256 x 512).  The ladder runs to A^32 only (A^64 is
applied as two bf16 A^32 L-folds): A^2, A^4 bf16 product pairs, A^8..
A^32 fp8 DoubleRow product pairs -- pairs, not PE transposes, keep the
tensor queue in dense 512-wide streams (transposes serialize LDWEIGHTS
and the resulting gaps drop the PE out of its max p-state).  Evictions
rotate across vector/scalar/gpsimd so they never stall the PE.  The
R-chain, S-matmul, L-folds and the final z-fold are all bf16.  No
Horner chain, no collective; all 8 cores run the identical replicated
program and the host takes core 0's z.
"""

import numpy as np
import ml_dtypes

import concourse.bacc as bacc
import concourse.mybir as mybir
from concourse.bass_utils import run_bass_kernel_spmd
from concourse.tile import TileContext
from concourse.masks import make_identity

N = 512
MC = 16
T = 128           # truncated scan length
S = 16            # stride: t = rho + S*q
NQ = T // S       # 8 L-factors
BATCH = 64
N_CORES = 8
KT = N // 128     # 4 contraction tiles
BF = mybir.dt.bfloat16
F32 = mybir.dt.float32
F8 = mybir.dt.float8e4
DR = mybir.MatmulPerfMode.DoubleRow
AF = mybir.ActivationFunctionType

# fp8 carry scales per stored power (power-of-2; keep max entry ~60-80)
S4 = 512.0
S8 = 512.0
S16 = 512.0
SR = 512.0        # Rcat^T fp8 carry scale
SU = 16.0         # Uarr fp8 carry scale (applied on host)

_COMPILED = {}


def _build_nc():
    nc = bacc.Bacc("TRN2", target_bir_lowering=False)

    d_A = nc.dram_tensor("Amat", (128, KT, N), BF, kind="ExternalInput")
    d_AT = nc.dram_tensor("ATmat", (128, KT, N), BF, kind="ExternalInput")
    d_B = nc.dram_tensor("Bk", (128, KT, MC), BF, kind="ExternalInput")
    d_P = nc.dram_tensor("PnT", (128, KT, MC), BF, kind="ExternalInput")
    d_U = nc.dram_tensor("Uarr", (128, 2, NQ * BATCH), F8,
                         kind="ExternalInput")
    d_out = nc.dram_tensor("uT", (MC, BATCH), F32, kind="ExternalOutput")

    with TileContext(nc) as tc:
        with tc.tile_pool(name="w", bufs=1) as wp, \
             tc.tile_pool(name="pp", bufs=1, space="PSUM") as pp, \
             tc.tile_pool(name="pt", bufs=1, space="PSUM") as pt, \
             tc.tile_pool(name="pz", bufs=1, space="PSUM") as pz:

            def wtile(name, shape, dt=BF):
                return wp.tile(shape, dt, tag=name, name=name)

            t_A = wtile("A", [128, KT, N])
            t_AT = wtile("AT", [128, KT, N])
            t_I32 = wtile("I32", [128, 128], F32)
            t_Ib = wtile("Ib", [128, 128], BF)
            t_U = wtile("U", [128, 2, NQ * BATCH], F8)
            t_R = wtile("R", [128, KT, S * MC])      # [B_0..B_15] bf16
            t_RT = wtile("RT", [128, 2, N], F8)      # Rcat^T (x SR)
            t_S = wtile("Smat", [128, KT, NQ * BATCH])
            # L-slots: 0..7 = L_q^T; 8..11 = temp (L_q A^32 for q<4)
            t_L = wtile("Lc", [128, KT, 12, MC])

            t_X2 = wtile("X2", [128, KT, N])
            t_XT2 = wtile("XT2", [128, KT, N])
            t_X4f = wtile("X4f", [128, KT, N], F8)
            t_XT4 = wtile("XT4", [128, KT, N])
            t_XT4f = wtile("XT4f", [128, KT, N], F8)
            t_X8f = wtile("X8f", [128, KT, N], F8)
            t_XT8 = wtile("XT8", [128, KT, N])
            t_XT8f = wtile("XT8f", [128, KT, N], F8)
            t_X16 = wtile("X16", [128, KT, N])
            t_X16f = wtile("X16f", [128, KT, N], F8)
            t_XT16f = wtile("XT16f", [128, KT, N], F8)
            t_X32 = wtile("X32", [128, KT, N])

            # input DMA; tiny B/Pn first (the R-chain interleaves into
            # the first product), then A/AT k-chunks, then U (needed last)
            nc.sync.dma_start(out=t_R[:, :, 0:MC], in_=d_B[:])
            nc.sync.dma_start(out=t_L[:, :, 0, :], in_=d_P[:])
            for k in range(KT):
                nc.sync.dma_start(out=t_AT[:, k, :], in_=d_AT[:, k, :])
                nc.sync.dma_start(out=t_A[:, k, :], in_=d_A[:, k, :])
            nc.sync.dma_start(out=t_U[:], in_=d_U[:])

            # identities (on-device, no DMA dep)
            make_identity(nc, t_I32[:])
            nc.vector.tensor_copy(out=t_Ib[:], in_=t_I32[:])

            # PE clock-ramp warmup covering the whole input DMA window
            for wi in range(38):
                wps = pp.tile([128, N], F32, tag="pp", bufs=5,
                              name=f"warm_{wi}")
                nc.tensor.transpose(wps[:, 0:128], t_I32[:], t_I32[:])

            # eviction engines round-robin so the PE never waits on one
            ectr = [0]

            def ev(dst, src, scale=None):
                e = ectr[0] % 2
                ectr[0] += 1
                if e == 0:
                    if scale is None:
                        nc.vector.tensor_copy(out=dst, in_=src)
                    else:
                        nc.vector.tensor_scalar_mul(dst, src, scale)
                elif e == 1:
                    if scale is None:
                        nc.scalar.activation(dst, src, AF.Copy)
                    else:
                        nc.scalar.activation(dst, src, AF.Copy, scale=scale)
                else:
                    if scale is None:
                        nc.gpsimd.tensor_copy(out=dst, in_=src)
                    else:
                        nc.gpsimd.tensor_scalar_mul(dst, src, scale)

            def prodchunks(lhsT_t, rhs_t, pname, outs, dr):
                """4 per-m-block thunks of a 512^3 product (bf16 4-pass or
                fp8 DR 2-pass); outs = [(tile, scale|None), ...]"""
                def mk(m):
                    def th():
                        ps = pp.tile([128, N], F32, tag="pp", bufs=5,
                                     name=f"pp_{pname}_{m}")
                        if dr:
                            for p in range(2):
                                nc.tensor.matmul(
                                    ps[:],
                                    lhsT_t[:, 2 * p:2 * p + 2,
                                           128 * m:128 * (m + 1)],
                                    rhs_t[:, 2 * p:2 * p + 2, :],
                                    start=(p == 0), stop=(p == 1),
                                    perf_mode=DR)
                        else:
                            for k in range(KT):
                                nc.tensor.matmul(
                                    ps[:],
                                    lhsT_t[:, k, 128 * m:128 * (m + 1)],
                                    rhs_t[:, k, :],
                                    start=(k == 0), stop=(k == KT - 1))
                        for (ft, fs) in outs:
                            ev(ft[:, m, :], ps[:], fs)
                    return th
                return [mk(m) for m in range(KT)]

            def rchunks(lhsT_t, w, pname):
                """R-chain doubling: cols [w:2w] = A^k @ cols [0:w]"""
                def mk(m):
                    def th():
                        pr = pp.tile([128, N], F32, tag="pp", bufs=5,
                                     name=f"pr_{pname}_{m}")
                        for k in range(KT):
                            nc.tensor.matmul(
                                pr[:, 0:w],
                                lhsT_t[:, k, 128 * m:128 * (m + 1)],
                                t_R[:, k, 0:w],
                                start=(k == 0), stop=(k == KT - 1))
                        ev(t_R[:, m, w:2 * w], pr[:, 0:w])
                    return th
                return [mk(m) for m in range(KT)]

            def lchunks(lhsT_t, src0, w, dst0, pname):
                """L-fold: slots [dst0:dst0+w] = lhsT^T @ slots [src0:+w]"""
                def mk(m):
                    def th():
                        pr = pp.tile([128, N], F32, tag="pp", bufs=5,
                                     name=f"pl_{pname}_{m}")
                        for k in range(KT):
                            nc.tensor.matmul(
                                pr[:, 0:w * MC],
                                lhsT_t[:, k, 128 * m:128 * (m + 1)],
                                t_L[:, k, src0:src0 + w, :],
                                start=(k == 0), stop=(k == KT - 1))
                        ev(t_L[:, m, dst0:dst0 + w, :], pr[:, 0:w * MC])
                    return th
                return [mk(m) for m in range(KT)]

            def rtchunks():
                """Rcat^T via PE transposes, evicted fp8 (x SR)"""
                def mk(nb):
                    def th():
                        tp = pt.tile([128, 4, 128], BF, tag="pt4", bufs=2,
                                     name=f"rt_{nb}")
                        for cb in range(2):
                            nc.tensor.transpose(
                                tp[:, cb, :],
                                t_R[:, nb, 128 * cb:128 * (cb + 1)],
                                t_Ib[:])
                        ev(t_RT[:, :, 128 * nb:128 * (nb + 1)],
                           tp[:, 0:2, :], SR)
                    return th
                return [mk(nb) for nb in range(KT)]

            def smmchunks():
                """S = Rcat @ Uarr  (fp8 DR, contraction 256 in one pass)"""
                def mk(m):
                    def th():
                        ps = pp.tile([128, NQ * BATCH], F32, tag="pp",
                                     bufs=5, name=f"smm_{m}")
                        nc.tensor.matmul(
                            ps[:], t_RT[:, 0:2, 128 * m:128 * (m + 1)],
                            t_U[:, 0:2, :],
                            start=True, stop=True, perf_mode=DR)
                        ev(t_S[:, m, :], ps[:], 1.0 / (SR * SU))
                    return th
                return [mk(m) for m in range(KT)]

            def zip_emit(big, small):
                """big[0] small[0] big[1] small[1] ... ; keeps PE
                utilization high so the DVFS never downclocks"""
                for i in range(max(len(big), len(small))):
                    if i < len(big):
                        big[i]()
                    if i < len(small):
                        small[i]()

            def run(chunks):
                for th in chunks:
                    th()

            # ---- ladder pairs with small bursts between ----
            run(prodchunks(t_AT, t_A, "x2", [(t_X2, None)], False))
            run(prodchunks(t_A, t_AT, "t2", [(t_XT2, None)], False))
            run(rchunks(t_AT, MC, "r1"))                        # B_1
            run(prodchunks(t_XT2, t_X2, "x4", [(t_X4f, S4)], False))
            run(prodchunks(t_X2, t_XT2, "t4",
                           [(t_XT4, None), (t_XT4f, S4)], False))
            run(rchunks(t_XT2, 2 * MC, "r2"))                   # B_2,B_3
            run(prodchunks(t_XT4f, t_X4f, "x8",
                           [(t_X8f, S8 / (S4 * S4))], True))
            run(prodchunks(t_X4f, t_XT4f, "t8",
                           [(t_XT8, 1.0 / (S4 * S4)),
                            (t_XT8f, S8 / (S4 * S4))], True))
            run(prodchunks(t_XT8f, t_X8f, "x16",
                           [(t_X16, 1.0 / (S8 * S8)),
                            (t_X16f, S16 / (S8 * S8))], True))
            run(rchunks(t_XT4, 4 * MC, "r4"))                   # B_4..B_7
            run(prodchunks(t_X8f, t_XT8f, "t16",
                           [(t_XT16f, S16 / (S8 * S8))], True))
            run(rchunks(t_XT8, 8 * MC, "r8"))                   # B_8..B_15
            run(rtchunks())
            run(prodchunks(t_XT16f, t_X16f, "x32",
                           [(t_X32, 1.0 / (S16 * S16))], True))
            run(smmchunks())
            run(lchunks(t_X16, 0, 1, 1, "f1"))        # L_1 = L_0 A^16
            run(lchunks(t_X32, 0, 2, 2, "f2"))        # L_2,L_3
            run(lchunks(t_X32, 0, 4, 8, "f4a"))       # temp = L_{0..3} A^32
            # final: z = sum_q L_q S_q ; two halves (one pz bank, reused)
            t_u1 = wtile("u1", [MC, BATCH], F32)
            psa = pz.tile([MC, BATCH], F32, tag="pz", bufs=1, name="psa")
            i = 0
            for q in range(4):
                for nb in range(KT):
                    nc.tensor.matmul(
                        psa[:], t_L[:, nb, q, :],
                        t_S[:, nb, BATCH * q:BATCH * (q + 1)],
                        start=(i == 0), stop=(i == 15))
                    i += 1
            nc.scalar.activation(t_u1[:], psa[:], AF.Copy)
            for th in lchunks(t_X32, 8, 4, 4, "f4b"):  # L_{4..7}
                th()
            psb = pz.tile([MC, BATCH], F32, tag="pz", bufs=1, name="psb")
            i = 0
            for q in range(4, NQ):
                for nb in range(KT):
                    nc.tensor.matmul(
                        psb[:], t_L[:, nb, q, :],
                        t_S[:, nb, BATCH * q:BATCH * (q + 1)],
                        start=(i == 0), stop=(i == 15))
                    i += 1
            t_u = wtile("u", [MC, BATCH], F32)
            nc.vector.tensor_add(t_u[:], t_u1[:], psb[:])
            nc.sync.dma_start(out=d_out[:], in_=t_u[:])

    nc.compile()
    return nc


def _arr512(m, dtype=ml_dtypes.bfloat16):
    """(512, X) -> (128, 4, X) k-tiled partition layout."""
    x = m.shape[1]
    return np.ascontiguousarray(
        m.reshape(KT, 128, x).transpose(1, 0, 2)).astype(dtype)


def _prep_inputs(A, B, C, K, bias, M0, M_tensor, sigma_phi_m, sigma_phi_M,
                 u_hist_rev, y_nat_history, y_obs):
    bf = ml_dtypes.bfloat16
    A = np.asarray(A, np.float32)
    C = np.asarray(C, np.float32)
    B = np.asarray(B, np.float32)
    K = np.asarray(K, np.float32)
    U = np.asarray(u_hist_rev, np.float32)[..., 0]        # (64, 512, 16)
    ynh = np.asarray(y_nat_history, np.float32)[..., 0]   # (64, 20, 512)
    yo = np.asarray(y_obs, np.float32)[..., 0]            # (64, 512)

    s_m = np.asarray(sigma_phi_m, np.float32).sum(axis=1)
    W0 = np.einsum('chn,h->cn', np.asarray(M0, np.float32), s_m)
    D = np.einsum('cijn,ik,j->ckn', np.asarray(M_tensor, np.float32),
                  np.asarray(sigma_phi_M, np.float32), s_m)   # (16, 10, 512)
    G = W0 + D[:, 0]
    Pn = -(G @ C)                                   # (16, 512)
    Qall = -K + G

    # host constants: Qall yo + sum_{k>=1} D_k hist_k + bias   -> (64, 16)
    Yk = np.stack([ynh[:, 20 - k] for k in range(1, 10)], axis=1)  # (64,9,512)
    const = (yo @ Qall.T
             + np.einsum('ckn,bkn->bc', D[:, 1:], Yk)
             + np.asarray(bias, np.float32)[:, 0][None, :])

    # Uarr[(rho,c), (q,b)] = u[b, rho + S q, c];  contraction idx k-tiled
    Ut = U[:, :T, :].reshape(BATCH, NQ, S, MC)            # [b, q, rho, c]
    Ua = Ut.transpose(2, 3, 1, 0).reshape(S * MC, NQ * BATCH)
    Ua = Ua.reshape(2, 128, NQ * BATCH).transpose(1, 0, 2)

    f8 = ml_dtypes.float8_e4m3
    m = {
        "Amat": _arr512(A),
        "ATmat": _arr512(np.ascontiguousarray(A.T)),
        "Bk": _arr512(B),
        "PnT": _arr512(np.ascontiguousarray(Pn.T)),
        "Uarr": (np.ascontiguousarray(Ua) * SU).astype(f8),
    }
    return [m] * N_CORES, const


def _run(in_maps, **kwargs):
    if "nc" not in _COMPILED:
        _COMPILED["nc"] = _build_nc()
    return run_bass_kernel_spmd(
        _COMPILED["nc"], in_maps, core_ids=list(range(N_CORES)), **kwargs)


def kernel(A, B, C, K, bias, M0, M_tensor, sigma_phi_m, sigma_phi_M,
           u_hist_rev, y_nat_history, y_obs, _profile=False):
    in_maps, const = _prep_inputs(
        A, B, C, K, bias, M0, M_tensor, sigma_phi_m, sigma_phi_M,
        u_hist_rev, y_nat_history, y_obs)
    res = _run(in_maps, trace=_profile)
    z = res.results[0]["uT"].astype(np.float32)     # replicated; take core 0
    u = z.T + const
    out = u[..., None].astype(np.float32)           # (64, 16, 1)
    if _profile:
        return out, res
    return out
